# revision 24
# baseline (speedup 1.0000x reference)
"""MoE (top-2 of 8 experts, D=1024, F=4096, T=8192) on 8 TRN2 NeuronCores.

Strategy: expert-parallel with load-balanced spill. The router (~0.05% of
FLOPs) runs on host with jax-CPU so expert selection is bit-identical to
the reference. Core e owns expert e's first <=2048 tokens as 4 pure
512-token tiles; tokens beyond 2048 on overloaded experts are split into
<=64-token spill chunks, one per core, processed by a uniform extra
64-wide segment with its own weight streams. This lowers the per-core
PE load from ceil(max_count/128)*128 = 2176 tokens to 2048 + ~64
(64-wide matmuls sustain ~29ns thanks to the auto-enabled fast weight
load), near the 2112-unit structural floor for these counts.

Device kernel per core: stage 1 computes hT[f, tok] per 512-token tile
(8 d-chunk matmuls per 128-f-chunk PSUM bank, w1 stationary), fused
bias+relu to SBUF on alternating Scalar/Vector engines; stage 2 computes
outT[d, tok] with 8 d-chunk PSUM banks (w2 stationary), bias added while
draining, output stored fp16 (host upcasts). The last tile's stage 2
runs in two d-halves so the final drain + output DMA tail is short.
The spill segment's stage-1/2 work and weight-strip DMAs interleave
between main stage-1 iterations (~1 unit per 2 iters) so its 16.8 MB
weight stream never stalls the in-order PE queue; spill stage-2
accumulates 4-fc groups in a transient PSUM bank (8 64-wide windows)
and folds them into an SBUF fp32 accumulator on the Vector engine.

DMA-queue discipline matters: each DMA instruction occupies the sync
queue ~600ns regardless of size, so x tiles and outputs move as single
3D DMAs, w2 streams as full 256KB strips, and the next tile's first two
w1 strips prefetch during the previous stage 2 (kills boundary stalls).
Warm-up matmuls on zeroed tiles fill the startup-DMA window so the PE's
activity monitor reaches full clock before real work arrives.

Matmul operands are fp16 (fp32 PSUM accumulation, ~4e-4 rel err); fp32
native matmul runs 4-5x slower, fp8 measures ~6e-2 rel err (over the
2e-2 budget).
"""

import numpy as np

D_MODEL = 1024
D_FF = 4096
N_EXPERTS = 8
TOP_K = 2
N_CORES = 8
TILE_N = 512
N_TILES = 4
MAIN_CAP = N_TILES * TILE_N    # 2048
SPILL_W = 64
CT = MAIN_CAP + SPILL_W        # x / y columns per core
FC = D_FF // 128               # 32 f-chunks
DC = D_MODEL // 128            # 8 d-chunks
WARMUP_MMS = 8

TRACE = False
LAST_EXEC_NS = None
LAST_TRACE_PATH = None

_nc_cache = {}


class _Bg:
    """Queue of background emission units interleaved into main loops."""

    def __init__(self):
        self.units = []
        self.frac = 0.0

    def add(self, fn):
        self.units.append(fn)

    def tick(self, rate):
        self.frac += rate
        while self.frac >= 1.0 and self.units:
            self.frac -= 1.0
            self.units.pop(0)()

    def flush(self):
        while self.units:
            self.units.pop(0)()


def _build_nc():
    import concourse.bacc as bacc
    import concourse.tile as tile
    import concourse.mybir as mybir

    f32 = mybir.dt.float32
    f16 = mybir.dt.float16
    AFT = mybir.ActivationFunctionType

    nc = bacc.Bacc("TRN2", target_bir_lowering=False, debug=False,
                   num_devices=N_CORES)
    xp = nc.dram_tensor("xp", [D_MODEL, CT], f16, kind="ExternalInput").ap()
    w1p = nc.dram_tensor("w1p", [FC, 128, D_MODEL], f16,
                         kind="ExternalInput").ap()
    w2p = nc.dram_tensor("w2p", [D_FF, D_MODEL], f16,
                         kind="ExternalInput").ap()
    w1x = nc.dram_tensor("w1x", [FC, 128, D_MODEL], f16,
                         kind="ExternalInput").ap()
    w2x = nc.dram_tensor("w2x", [D_FF, D_MODEL], f16,
                         kind="ExternalInput").ap()
    b1p = nc.dram_tensor("b1p", [128, FC], f32, kind="ExternalInput").ap()
    b2p = nc.dram_tensor("b2p", [128, DC], f32, kind="ExternalInput").ap()
    b1xp = nc.dram_tensor("b1xp", [128, FC], f32, kind="ExternalInput").ap()
    b2xp = nc.dram_tensor("b2xp", [128, DC], f32, kind="ExternalInput").ap()
    yp = nc.dram_tensor("yp", [D_MODEL, CT], f16, kind="ExternalOutput").ap()

    xp_r = xp.rearrange("(c p) t -> p c t", p=128)   # [128, 8, CT]
    yp_r = yp.rearrange("(c p) t -> p c t", p=128)

    with tile.TileContext(nc) as tc:
        with (
            tc.tile_pool(name="const", bufs=1) as constp,
            tc.tile_pool(name="x", bufs=3) as xpool,
            tc.tile_pool(name="xx", bufs=1) as xxpool,
            tc.tile_pool(name="h", bufs=1) as hpool,
            tc.tile_pool(name="hx", bufs=1) as hxpool,
            tc.tile_pool(name="accx", bufs=1) as accxpool,
            tc.tile_pool(name="w1", bufs=6) as w1pool,
            tc.tile_pool(name="w2", bufs=12) as w2pool,
            tc.tile_pool(name="w1xs", bufs=4) as w1xpool,
            tc.tile_pool(name="w2xs", bufs=12) as w2xpool,
            tc.tile_pool(name="o", bufs=2) as opool,
            tc.tile_pool(name="ox", bufs=1) as oxpool,
            tc.tile_pool(name="ps", bufs=8, space="PSUM") as pspool,
        ):
            b1s = constp.tile([128, FC], f32)
            b2s = constp.tile([128, DC], f32)
            b1xs = constp.tile([128, FC], f32)
            b2xs = constp.tile([128, DC], f32)

            def load_xs(t0):
                xs = xpool.tile([128, DC, TILE_N], f16, tag="xs",
                                name=f"xs_{t0}")
                nc.sync.dma_start(xs[:, :, :], xp_r[:, :, t0:t0 + TILE_N])
                return xs

            def load_w1s(fc, eng=None):
                w1s = w1pool.tile([128, D_MODEL], f16, tag="w1s",
                                  name=f"w1s_{fc}")
                (eng or nc.sync).dma_start(w1s[:], w1p[fc])
                return w1s

            # ---- spill segment state ----
            hxs = hxpool.tile([128, FC * SPILL_W], f16, tag="hx")
            accx = accxpool.tile([128, DC * SPILL_W], f32, tag="accx")
            oxs = oxpool.tile([128, DC, SPILL_W], f16, tag="ox")
            w1x_tiles = {}
            w2x_tiles = {}

            def dma_w1x(fc):
                if fc >= FC:
                    return
                t = w1xpool.tile([128, D_MODEL], f16, tag="w1xs",
                                 name=f"w1xs_{fc}")
                nc.sync.dma_start(t[:], w1x[fc])
                w1x_tiles[fc] = t

            def dma_w2x_group(g):
                if g >= 8:
                    return
                for j in range(4):
                    fc = g * 4 + j
                    t = w2xpool.tile([128, D_MODEL], f16, tag="w2xs",
                                     name=f"w2xs_{fc}")
                    nc.sync.dma_start(t[:], w2x[fc * 128:(fc + 1) * 128, :])
                    w2x_tiles[fc] = t

            def spill_s1_unit(fc, xsx):
                def emit():
                    dma_w1x(fc + 2)
                    ps = pspool.tile([128, SPILL_W], f32, tag="ps",
                                     name=f"ps1x_{fc}")
                    w1t = w1x_tiles.pop(fc)
                    for c in range(DC):
                        nc.tensor.matmul(
                            ps[:],
                            lhsT=w1t[:, c * 128:(c + 1) * 128],
                            rhs=xsx[:, c, :],
                            start=(c == 0),
                            stop=(c == DC - 1),
                        )
                    dst = hxs[:, fc * SPILL_W:(fc + 1) * SPILL_W]
                    if fc % 2 == 0:
                        nc.scalar.activation(dst, ps[:], AFT.Relu,
                                             bias=b1xs[:, fc:fc + 1])
                    else:
                        nc.vector.tensor_scalar(
                            dst, ps[:], b1xs[:, fc:fc + 1], 0.0,
                            mybir.AluOpType.add, mybir.AluOpType.max)
                return emit

            def spill_s2_unit(g):
                def emit():
                    dma_w2x_group(g + 2)
                    # per-dc transient bank with a standard 4-matmul
                    # accumulation group, folded into the SBUF accumulator
                    for dc in range(DC):
                        tmp = pspool.tile([128, SPILL_W], f32, tag="ps",
                                          name=f"ps2x_{g}_{dc}")
                        for j in range(4):
                            fc = g * 4 + j
                            nc.tensor.matmul(
                                tmp[:],
                                lhsT=w2x_tiles[fc][:, dc * 128:(dc + 1) * 128],
                                rhs=hxs[:, fc * SPILL_W:(fc + 1) * SPILL_W],
                                start=(j == 0),
                                stop=(j == 3),
                            )
                        dst = accx[:, dc * SPILL_W:(dc + 1) * SPILL_W]
                        if g == 0:
                            nc.vector.tensor_copy(dst, tmp[:])
                        else:
                            nc.vector.tensor_add(dst, dst, tmp[:])
                    for j in range(4):
                        w2x_tiles.pop(g * 4 + j)
                return emit

            def spill_drain():
                for dc in range(DC):
                    src = accx[:, dc * SPILL_W:(dc + 1) * SPILL_W]
                    dst = oxs[:, dc, :]
                    if dc % 2 == 0:
                        nc.vector.tensor_scalar_add(dst, src,
                                                    b2xs[:, dc:dc + 1])
                    else:
                        nc.scalar.activation(dst, src, AFT.Identity,
                                             bias=b2xs[:, dc:dc + 1])
                nc.sync.dma_start(
                    yp_r[:, :, MAIN_CAP:MAIN_CAP + SPILL_W], oxs[:, :, :])

            bg = _Bg()

            def stage1(xs, h, t0, pre=None, bg_rate=0.0):
                for fc in range(FC):
                    w1s = pre[fc] if pre and fc in pre else load_w1s(fc)
                    ps = pspool.tile([128, TILE_N], f32, tag="ps",
                                     name=f"ps_{t0}_{fc}")
                    for c in range(DC):
                        nc.tensor.matmul(
                            ps[:],
                            lhsT=w1s[:, c * 128:(c + 1) * 128],
                            rhs=xs[:, c, :],
                            start=(c == 0),
                            stop=(c == DC - 1),
                        )
                    dst = h[:, fc * TILE_N:(fc + 1) * TILE_N]
                    # alternate relu between Scalar and Vector engines so
                    # consecutive psum banks release in parallel
                    if fc % 2 == 0:
                        nc.scalar.activation(dst, ps[:], AFT.Relu,
                                             bias=b1s[:, fc:fc + 1])
                    else:
                        nc.vector.tensor_scalar(
                            dst, ps[:], b1s[:, fc:fc + 1], 0.0,
                            mybir.AluOpType.add, mybir.AluOpType.max)
                    bg.tick(bg_rate)

            def stage2_full(h, t0, hooks=None):
                ps2 = [pspool.tile([128, TILE_N], f32, tag="ps",
                                   name=f"ps2_{t0}_{dc}")
                       for dc in range(DC)]
                outs = opool.tile([128, DC, TILE_N], f16, tag="o",
                                  name=f"outs_{t0}")
                for fc in range(FC):
                    w2s = w2pool.tile([128, D_MODEL], f16, tag="w2s",
                                      name=f"w2s_{t0}_{fc}")
                    nc.sync.dma_start(w2s[:], w2p[fc * 128:(fc + 1) * 128, :])
                    for dc in range(DC):
                        nc.tensor.matmul(
                            ps2[dc][:],
                            lhsT=w2s[:, dc * 128:(dc + 1) * 128],
                            rhs=h[:, fc * TILE_N:(fc + 1) * TILE_N],
                            start=(fc == 0),
                            stop=(fc == FC - 1),
                        )
                    if hooks and fc in hooks:
                        hooks[fc]()
                for dc in range(DC):
                    dst = outs[:, dc, :]
                    if dc % 2 == 0:
                        nc.vector.tensor_scalar_add(dst, ps2[dc][:],
                                                    b2s[:, dc:dc + 1])
                    else:
                        nc.scalar.activation(dst, ps2[dc][:], AFT.Identity,
                                             bias=b2s[:, dc:dc + 1])
                nc.sync.dma_start(yp_r[:, :, t0:t0 + TILE_N], outs[:, :, :])

            def stage2_halves(h, t0):
                # last tile: two d-halves so the final drain + out DMA is
                # only 4 banks / 0.5 MB
                for half in range(2):
                    ps2 = [pspool.tile([128, TILE_N], f32, tag="ps",
                                       name=f"ps2h_{half}_{j}")
                           for j in range(4)]
                    outs = opool.tile([128, 4, TILE_N], f16, tag="o",
                                      name=f"outsh_{half}")
                    for fc in range(FC):
                        w2s = w2pool.tile([128, 512], f16, tag="w2s",
                                          name=f"w2sh_{half}_{fc}")
                        nc.sync.dma_start(
                            w2s[:],
                            w2p[fc * 128:(fc + 1) * 128,
                                half * 512:(half + 1) * 512],
                        )
                        for j in range(4):
                            nc.tensor.matmul(
                                ps2[j][:],
                                lhsT=w2s[:, j * 128:(j + 1) * 128],
                                rhs=h[:, fc * TILE_N:(fc + 1) * TILE_N],
                                start=(fc == 0),
                                stop=(fc == FC - 1),
                            )
                    # drain + DMA in dc pairs so the first pair's output
                    # transfer overlaps the second pair's drain (short tail)
                    for pair in range(2):
                        for j in (pair * 2, pair * 2 + 1):
                            dc = half * 4 + j
                            dst = outs[:, j, :]
                            if j % 2 == 0:
                                nc.vector.tensor_scalar_add(
                                    dst, ps2[j][:], b2s[:, dc:dc + 1])
                            else:
                                nc.scalar.activation(
                                    dst, ps2[j][:], AFT.Identity,
                                    bias=b2s[:, dc:dc + 1])
                        d0 = half * 4 + pair * 2
                        nc.sync.dma_start(
                            yp_r[:, d0:d0 + 2, t0:t0 + TILE_N],
                            outs[:, pair * 2:pair * 2 + 2, :])

            # Warm-up: dummy matmuls on zeroed tiles fill the otherwise
            # idle startup-DMA window so the PE's activity monitor (HAM)
            # reaches full clock before real work arrives.
            warm_w = w1pool.tile([128, 128], f16, tag="warmw")
            warm_x = xxpool.tile([128, 512], f16, tag="warmx")
            nc.gpsimd.memset(warm_w[:], 0.0)
            nc.gpsimd.memset(warm_x[:], 0.0)
            warm_ps = pspool.tile([128, 512], f32, tag="ps", name="warm_ps")
            for _ in range(WARMUP_MMS):
                nc.tensor.matmul(warm_ps[:], lhsT=warm_w[:], rhs=warm_x[:],
                                 start=True, stop=True)

            # head: tile-0 x loads per-chunk so the first matmul can start
            # as soon as chunk 0 lands (one 3D DMA would gate fc=0 on the
            # whole 1MB tile); w1 strips 0-5 hoisted; then spill x
            # tile-0 x chunks ride the otherwise-idle scalar/gpsimd DGE
            # queues in parallel with the w1 strips on the sync queue, so
            # the first stage-1 iteration is fed ~4us sooner; bias
            # constants follow on the same side queues
            xs = xpool.tile([128, DC, TILE_N], f16, tag="xs", name="xs_0")
            # w1 strip 0 on the scalar queue so the sync queue's first
            # slots carry x chunks (x supply gates stage-1 fc=0)
            pre = {0: load_w1s(0, eng=nc.scalar)}
            for c in range(DC):
                # gpsimd first: the scalar queue opens late (blocked by
                # the ~1.3us activation-table load in the preamble)
                eng = (nc.gpsimd, nc.sync, nc.scalar)[c % 3]
                eng.dma_start(xs[:, c, :], xp_r[:, c, 0:TILE_N])
            nc.scalar.dma_start(b1s[:], b1p)
            nc.gpsimd.dma_start(b2s[:], b2p)
            nc.scalar.dma_start(b1xs[:], b1xp)
            nc.gpsimd.dma_start(b2xs[:], b2xp)
            for fc in range(1, 6):
                pre[fc] = load_w1s(fc)
            xsx = xxpool.tile([128, DC, SPILL_W], f16, tag="xsx")
            nc.sync.dma_start(xsx[:, :, :],
                              xp_r[:, :, MAIN_CAP:MAIN_CAP + SPILL_W])

            # background queue: spill stage-1 units (w1x strips prefetched
            # 2 ahead), then spill stage-2 group units (strips 2 groups
            # ahead); emitted during stage-1 of tiles 1-3
            bg.add(lambda: dma_w1x(0))
            bg.add(lambda: dma_w1x(1))
            for fc in range(FC):
                bg.add(spill_s1_unit(fc, xsx))
            bg.add(lambda: dma_w2x_group(0))
            bg.add(lambda: dma_w2x_group(1))
            for g in range(8):
                bg.add(spill_s2_unit(g))

            nxt = {}

            def hook_prefetch(i):
                def fx():
                    nxt['xs'] = load_xs(i * TILE_N)
                def fw():
                    nxt['pre'] = {0: load_w1s(0), 1: load_w1s(1)}
                return {8: fx, 16: fw}

            for i in range(N_TILES):
                t0 = i * TILE_N
                h = hpool.tile([128, FC * TILE_N], f16, tag="h",
                               name=f"h_{i}")
                stage1(xs, h, t0, pre=pre, bg_rate=0.0 if i == 0 else 0.5)
                if i == N_TILES - 1:
                    bg.flush()
                    spill_drain()
                    stage2_halves(h, t0)
                else:
                    stage2_full(h, t0, hooks=hook_prefetch(i + 1))
                    xs = nxt.pop('xs')
                    pre = nxt.pop('pre')

    nc.compile()
    return nc


def _ensure_trace_hook():
    """bass_utils' axon trace path needs antenv.axon_hooks; inject it."""
    import sys
    import types
    try:
        import antenv
        if "antenv.axon_hooks" in sys.modules:
            return
        from trn_agent_boot.trn_boot import _ntff_profile_via_ctypes
        mod = types.ModuleType("antenv.axon_hooks")
        hook = [_ntff_profile_via_ctypes("/opt/axon/libaxon_pjrt.so")]
        mod.set_axon_ntff_profile_hook = lambda h: hook.__setitem__(0, h)
        mod.get_axon_ntff_profile_hook = lambda: hook[0]
        sys.modules["antenv.axon_hooks"] = mod
        antenv.axon_hooks = mod
    except Exception:
        pass


def _route(xf, router_w, router_b):
    """Top-2 routing, bit-identical to the reference (jax on CPU)."""
    try:
        import jax
        import jax.numpy as jnp

        cpu = jax.devices("cpu")[0]
        with jax.default_device(cpu):
            logits = (jnp.asarray(xf) @ jnp.asarray(router_w)
                      + jnp.asarray(router_b))
            top_vals, top_idx = jax.lax.top_k(logits, TOP_K)
            wts = jax.nn.softmax(top_vals, axis=-1)
        return np.asarray(top_idx), np.asarray(wts, np.float32)
    except Exception:
        # numpy fallback; ties resolve to the lower index like lax.top_k
        logits = xf @ router_w + router_b
        order = np.argsort(-logits, axis=1, kind="stable")[:, :TOP_K]
        vals = np.take_along_axis(logits, order, axis=1)
        ex = np.exp(vals - vals.max(axis=1, keepdims=True))
        wts = (ex / ex.sum(axis=1, keepdims=True)).astype(np.float32)
        return order, wts


def _w1_layout(w):
    """[D_MODEL, D_FF] -> [FC, 128, D_MODEL] strips for lhsT slicing."""
    return np.ascontiguousarray(
        w.reshape(DC, 128, FC, 128).transpose(2, 1, 0, 3)
        .reshape(FC, 128, D_MODEL).astype(np.float16))


def kernel(x, router_w, router_b, w1, b1, w2, b2):
    global LAST_EXEC_NS, LAST_TRACE_PATH
    from concourse import bass_utils

    x = np.asarray(x, np.float32)
    router_w = np.asarray(router_w, np.float32)
    router_b = np.asarray(router_b, np.float32)
    w1 = np.asarray(w1, np.float32)
    b1 = np.asarray(b1, np.float32)
    w2 = np.asarray(w2, np.float32)
    b2 = np.asarray(b2, np.float32)

    orig_shape = x.shape
    xf = x.reshape(-1, x.shape[-1])
    T = xf.shape[0]

    top_idx, wts = _route(xf, router_w, router_b)

    tok_ids = []
    gates = []
    for e in range(N_EXPERTS):
        mask = top_idx == e                      # [T, K]
        sel = mask.any(axis=1)
        ids = np.nonzero(sel)[0]
        # each token picks distinct experts, so at most one k matches
        gk = np.where(mask[ids, 0], wts[ids, 0], wts[ids, 1]).astype(np.float32)
        tok_ids.append(ids)
        gates.append(gk)

    counts = np.array([len(i) for i in tok_ids])

    # spill chunks: tokens beyond MAIN_CAP, split into <=SPILL_W chunks,
    # one chunk per core
    spill_chunks = []           # (expert, ids, gates)
    for e in range(N_EXPERTS):
        rem = counts[e] - MAIN_CAP
        off = MAIN_CAP
        while rem > 0:
            n = min(rem, SPILL_W)
            spill_chunks.append((e, tok_ids[e][off:off + n],
                                 gates[e][off:off + n]))
            off += n
            rem -= n
    assert len(spill_chunks) <= N_CORES, (
        f"spill overflow: {len(spill_chunks)} chunks > {N_CORES} cores")

    key = "v3"
    if key not in _nc_cache:
        _nc_cache[key] = _build_nc()
    nc = _nc_cache[key]

    xfT = xf.astype(np.float16)
    w2_f16 = [np.ascontiguousarray(w2[e].astype(np.float16))
              for e in range(N_EXPERTS)]
    w1_lay = [_w1_layout(w1[e]) for e in range(N_EXPERTS)]
    in_maps = []
    for e in range(N_EXPERTS):
        ce = min(counts[e], MAIN_CAP)
        xpad = np.zeros((D_MODEL, CT), np.float16)
        xpad[:, :ce] = xfT[tok_ids[e][:ce]].T
        if e < len(spill_chunks):
            se, sids = spill_chunks[e][0], spill_chunks[e][1]
        else:
            se, sids = e, np.empty(0, np.int64)
        if len(sids):
            xpad[:, MAIN_CAP:MAIN_CAP + len(sids)] = xfT[sids].T
        in_maps.append({
            "xp": xpad,
            "w1p": w1_lay[e],
            "w2p": w2_f16[e],
            "w1x": w1_lay[se],
            "w2x": w2_f16[se],
            "b1p": np.ascontiguousarray(b1[e].reshape(FC, 128).T),
            "b2p": np.ascontiguousarray(b2[e].reshape(DC, 128).T),
            "b1xp": np.ascontiguousarray(b1[se].reshape(FC, 128).T),
            "b2xp": np.ascontiguousarray(b2[se].reshape(DC, 128).T),
        })

    if TRACE:
        _ensure_trace_hook()
    res = bass_utils.run_bass_kernel_spmd(
        nc, in_maps, core_ids=list(range(N_CORES)), trace=TRACE)
    LAST_EXEC_NS = res.exec_time_ns
    LAST_TRACE_PATH = (res.instructions_and_trace[1]
                       if res.instructions_and_trace else None)

    out = np.zeros((T, D_MODEL), np.float32)
    for e in range(N_EXPERTS):
        ye = np.asarray(res.results[e]["yp"]).astype(np.float32)  # [D, CT]
        ce = min(counts[e], MAIN_CAP)
        out[tok_ids[e][:ce]] += gates[e][:ce, None] * ye.T[:ce]
        if e < len(spill_chunks):
            se, sids, sg = spill_chunks[e]
            if len(sids):
                out[sids] += sg[:, None] * ye.T[MAIN_CAP:MAIN_CAP + len(sids)]

    return out.reshape(orig_shape)


# revision 25
# speedup vs baseline: 1.0016x; 1.0016x over previous
"""MoE (top-2 of 8 experts, D=1024, F=4096, T=8192) on 8 TRN2 NeuronCores.

Strategy: expert-parallel with load-balanced spill. The router (~0.05% of
FLOPs) runs on host with jax-CPU so expert selection is bit-identical to
the reference. Core e owns expert e's first <=2048 tokens as 4 pure
512-token tiles; tokens beyond 2048 on overloaded experts are split into
<=64-token spill chunks, one per core, processed by a uniform extra
64-wide segment with its own weight streams. This lowers the per-core
PE load from ceil(max_count/128)*128 = 2176 tokens to 2048 + ~64
(64-wide matmuls sustain ~29ns thanks to the auto-enabled fast weight
load), near the 2112-unit structural floor for these counts.

Device kernel per core: stage 1 computes hT[f, tok] per 512-token tile
(8 d-chunk matmuls per 128-f-chunk PSUM bank, w1 stationary), fused
bias+relu to SBUF on alternating Scalar/Vector engines; stage 2 computes
outT[d, tok] with 8 d-chunk PSUM banks (w2 stationary), bias added while
draining, output stored fp16 (host upcasts). The last tile's stage 2
runs in two d-halves so the final drain + output DMA tail is short.
The spill segment's stage-1/2 work and weight-strip DMAs interleave
between main stage-1 iterations (~1 unit per 2 iters) so its 16.8 MB
weight stream never stalls the in-order PE queue; spill stage-2
accumulates 4-fc groups in a transient PSUM bank (8 64-wide windows)
and folds them into an SBUF fp32 accumulator on the Vector engine.

DMA-queue discipline matters: each DMA instruction occupies the sync
queue ~600ns regardless of size, so x tiles and outputs move as single
3D DMAs, w2 streams as full 256KB strips, and the next tile's first two
w1 strips prefetch during the previous stage 2 (kills boundary stalls).
Warm-up matmuls on zeroed tiles fill the startup-DMA window so the PE's
activity monitor reaches full clock before real work arrives.

Matmul operands are fp16 (fp32 PSUM accumulation, ~4e-4 rel err); fp32
native matmul runs 4-5x slower, fp8 measures ~6e-2 rel err (over the
2e-2 budget).
"""

import numpy as np

D_MODEL = 1024
D_FF = 4096
N_EXPERTS = 8
TOP_K = 2
N_CORES = 8
TILE_N = 512
N_TILES = 4
MAIN_CAP = N_TILES * TILE_N    # 2048
SPILL_W = 64
CT = MAIN_CAP + SPILL_W        # x / y columns per core
FC = D_FF // 128               # 32 f-chunks
DC = D_MODEL // 128            # 8 d-chunks
WARMUP_MMS = 8

TRACE = False
LAST_EXEC_NS = None
LAST_TRACE_PATH = None

_nc_cache = {}


class _Bg:
    """Queue of background emission units interleaved into main loops."""

    def __init__(self):
        self.units = []
        self.frac = 0.0

    def add(self, fn):
        self.units.append(fn)

    def tick(self, rate):
        self.frac += rate
        while self.frac >= 1.0 and self.units:
            self.frac -= 1.0
            self.units.pop(0)()

    def flush(self):
        while self.units:
            self.units.pop(0)()


def _build_nc():
    import concourse.bacc as bacc
    import concourse.tile as tile
    import concourse.mybir as mybir

    f32 = mybir.dt.float32
    f16 = mybir.dt.float16
    AFT = mybir.ActivationFunctionType

    nc = bacc.Bacc("TRN2", target_bir_lowering=False, debug=False,
                   num_devices=N_CORES)
    xp = nc.dram_tensor("xp", [D_MODEL, CT], f16, kind="ExternalInput").ap()
    w1p = nc.dram_tensor("w1p", [FC, 128, D_MODEL], f16,
                         kind="ExternalInput").ap()
    w2p = nc.dram_tensor("w2p", [D_FF, D_MODEL], f16,
                         kind="ExternalInput").ap()
    w1x = nc.dram_tensor("w1x", [FC, 128, D_MODEL], f16,
                         kind="ExternalInput").ap()
    w2x = nc.dram_tensor("w2x", [D_FF, D_MODEL], f16,
                         kind="ExternalInput").ap()
    b1p = nc.dram_tensor("b1p", [128, FC], f32, kind="ExternalInput").ap()
    b2p = nc.dram_tensor("b2p", [128, DC], f32, kind="ExternalInput").ap()
    b1xp = nc.dram_tensor("b1xp", [128, FC], f32, kind="ExternalInput").ap()
    b2xp = nc.dram_tensor("b2xp", [128, DC], f32, kind="ExternalInput").ap()
    yp = nc.dram_tensor("yp", [D_MODEL, CT], f16, kind="ExternalOutput").ap()

    xp_r = xp.rearrange("(c p) t -> p c t", p=128)   # [128, 8, CT]
    yp_r = yp.rearrange("(c p) t -> p c t", p=128)

    with tile.TileContext(nc) as tc:
        with (
            tc.tile_pool(name="const", bufs=1) as constp,
            tc.tile_pool(name="x", bufs=3) as xpool,
            tc.tile_pool(name="xx", bufs=1) as xxpool,
            tc.tile_pool(name="h", bufs=1) as hpool,
            tc.tile_pool(name="hx", bufs=1) as hxpool,
            tc.tile_pool(name="accx", bufs=1) as accxpool,
            tc.tile_pool(name="w1", bufs=6) as w1pool,
            tc.tile_pool(name="w2", bufs=12) as w2pool,
            tc.tile_pool(name="w1xs", bufs=4) as w1xpool,
            tc.tile_pool(name="w2xs", bufs=12) as w2xpool,
            tc.tile_pool(name="o", bufs=2) as opool,
            tc.tile_pool(name="ox", bufs=1) as oxpool,
            tc.tile_pool(name="ps", bufs=8, space="PSUM") as pspool,
        ):
            b1s = constp.tile([128, FC], f32)
            b2s = constp.tile([128, DC], f32)
            b1xs = constp.tile([128, FC], f32)
            b2xs = constp.tile([128, DC], f32)

            def load_xs(t0):
                xs = xpool.tile([128, DC, TILE_N], f16, tag="xs",
                                name=f"xs_{t0}")
                nc.sync.dma_start(xs[:, :, :], xp_r[:, :, t0:t0 + TILE_N])
                return xs

            def load_w1s(fc, eng=None):
                w1s = w1pool.tile([128, D_MODEL], f16, tag="w1s",
                                  name=f"w1s_{fc}")
                (eng or nc.sync).dma_start(w1s[:], w1p[fc])
                return w1s

            # ---- spill segment state ----
            hxs = hxpool.tile([128, FC * SPILL_W], f16, tag="hx")
            accx = accxpool.tile([128, DC * SPILL_W], f32, tag="accx")
            oxs = oxpool.tile([128, DC, SPILL_W], f16, tag="ox")
            w1x_tiles = {}
            w2x_tiles = {}

            def dma_w1x(fc):
                if fc >= FC:
                    return
                t = w1xpool.tile([128, D_MODEL], f16, tag="w1xs",
                                 name=f"w1xs_{fc}")
                nc.sync.dma_start(t[:], w1x[fc])
                w1x_tiles[fc] = t

            def dma_w2x_group(g):
                if g >= 8:
                    return
                for j in range(4):
                    fc = g * 4 + j
                    t = w2xpool.tile([128, D_MODEL], f16, tag="w2xs",
                                     name=f"w2xs_{fc}")
                    nc.sync.dma_start(t[:], w2x[fc * 128:(fc + 1) * 128, :])
                    w2x_tiles[fc] = t

            def spill_s1_unit(fc, xsx):
                def emit():
                    dma_w1x(fc + 2)
                    ps = pspool.tile([128, SPILL_W], f32, tag="ps",
                                     name=f"ps1x_{fc}")
                    w1t = w1x_tiles.pop(fc)
                    for c in range(DC):
                        nc.tensor.matmul(
                            ps[:],
                            lhsT=w1t[:, c * 128:(c + 1) * 128],
                            rhs=xsx[:, c, :],
                            start=(c == 0),
                            stop=(c == DC - 1),
                        )
                    dst = hxs[:, fc * SPILL_W:(fc + 1) * SPILL_W]
                    if fc % 2 == 0:
                        nc.scalar.activation(dst, ps[:], AFT.Relu,
                                             bias=b1xs[:, fc:fc + 1])
                    else:
                        nc.vector.tensor_scalar(
                            dst, ps[:], b1xs[:, fc:fc + 1], 0.0,
                            mybir.AluOpType.add, mybir.AluOpType.max)
                return emit

            def spill_s2_unit(g):
                def emit():
                    dma_w2x_group(g + 2)
                    # per-dc transient bank with a standard 4-matmul
                    # accumulation group, folded into the SBUF accumulator
                    for dc in range(DC):
                        tmp = pspool.tile([128, SPILL_W], f32, tag="ps",
                                          name=f"ps2x_{g}_{dc}")
                        for j in range(4):
                            fc = g * 4 + j
                            nc.tensor.matmul(
                                tmp[:],
                                lhsT=w2x_tiles[fc][:, dc * 128:(dc + 1) * 128],
                                rhs=hxs[:, fc * SPILL_W:(fc + 1) * SPILL_W],
                                start=(j == 0),
                                stop=(j == 3),
                            )
                        dst = accx[:, dc * SPILL_W:(dc + 1) * SPILL_W]
                        if g == 0:
                            nc.vector.tensor_copy(dst, tmp[:])
                        else:
                            nc.vector.tensor_add(dst, dst, tmp[:])
                    for j in range(4):
                        w2x_tiles.pop(g * 4 + j)
                return emit

            def spill_drain():
                for dc in range(DC):
                    src = accx[:, dc * SPILL_W:(dc + 1) * SPILL_W]
                    dst = oxs[:, dc, :]
                    if dc % 2 == 0:
                        nc.vector.tensor_scalar_add(dst, src,
                                                    b2xs[:, dc:dc + 1])
                    else:
                        nc.scalar.activation(dst, src, AFT.Identity,
                                             bias=b2xs[:, dc:dc + 1])
                nc.sync.dma_start(
                    yp_r[:, :, MAIN_CAP:MAIN_CAP + SPILL_W], oxs[:, :, :])

            bg = _Bg()

            def stage1(xs, h, t0, pre=None, bg_rate=0.0):
                for fc in range(FC):
                    w1s = pre[fc] if pre and fc in pre else load_w1s(fc)
                    ps = pspool.tile([128, TILE_N], f32, tag="ps",
                                     name=f"ps_{t0}_{fc}")
                    for c in range(DC):
                        nc.tensor.matmul(
                            ps[:],
                            lhsT=w1s[:, c * 128:(c + 1) * 128],
                            rhs=xs[:, c, :],
                            start=(c == 0),
                            stop=(c == DC - 1),
                        )
                    dst = h[:, fc * TILE_N:(fc + 1) * TILE_N]
                    # alternate relu between Scalar and Vector engines so
                    # consecutive psum banks release in parallel
                    if fc % 2 == 0:
                        nc.scalar.activation(dst, ps[:], AFT.Relu,
                                             bias=b1s[:, fc:fc + 1])
                    else:
                        nc.vector.tensor_scalar(
                            dst, ps[:], b1s[:, fc:fc + 1], 0.0,
                            mybir.AluOpType.add, mybir.AluOpType.max)
                    bg.tick(bg_rate)

            def stage2_full(h, t0, hooks=None):
                ps2 = [pspool.tile([128, TILE_N], f32, tag="ps",
                                   name=f"ps2_{t0}_{dc}")
                       for dc in range(DC)]
                outs = opool.tile([128, DC, TILE_N], f16, tag="o",
                                  name=f"outs_{t0}")
                for fc in range(FC):
                    w2s = w2pool.tile([128, D_MODEL], f16, tag="w2s",
                                      name=f"w2s_{t0}_{fc}")
                    nc.sync.dma_start(w2s[:], w2p[fc * 128:(fc + 1) * 128, :])
                    for dc in range(DC):
                        nc.tensor.matmul(
                            ps2[dc][:],
                            lhsT=w2s[:, dc * 128:(dc + 1) * 128],
                            rhs=h[:, fc * TILE_N:(fc + 1) * TILE_N],
                            start=(fc == 0),
                            stop=(fc == FC - 1),
                        )
                    if hooks and fc in hooks:
                        hooks[fc]()
                for dc in range(DC):
                    dst = outs[:, dc, :]
                    if dc % 2 == 0:
                        nc.vector.tensor_scalar_add(dst, ps2[dc][:],
                                                    b2s[:, dc:dc + 1])
                    else:
                        nc.scalar.activation(dst, ps2[dc][:], AFT.Identity,
                                             bias=b2s[:, dc:dc + 1])
                nc.sync.dma_start(yp_r[:, :, t0:t0 + TILE_N], outs[:, :, :])

            def stage2_halves(h, t0):
                # last tile: two d-halves so the final drain + out DMA is
                # only 4 banks / 0.5 MB
                for half in range(2):
                    ps2 = [pspool.tile([128, TILE_N], f32, tag="ps",
                                       name=f"ps2h_{half}_{j}")
                           for j in range(4)]
                    outs = opool.tile([128, 4, TILE_N], f16, tag="o",
                                      name=f"outsh_{half}")
                    for fc in range(FC):
                        w2s = w2pool.tile([128, 512], f16, tag="w2s",
                                          name=f"w2sh_{half}_{fc}")
                        nc.sync.dma_start(
                            w2s[:],
                            w2p[fc * 128:(fc + 1) * 128,
                                half * 512:(half + 1) * 512],
                        )
                        for j in range(4):
                            nc.tensor.matmul(
                                ps2[j][:],
                                lhsT=w2s[:, j * 128:(j + 1) * 128],
                                rhs=h[:, fc * TILE_N:(fc + 1) * TILE_N],
                                start=(fc == 0),
                                stop=(fc == FC - 1),
                            )
                    # drain + DMA in dc pairs so the first pair's output
                    # transfer overlaps the second pair's drain (short tail)
                    for pair in range(2):
                        for j in (pair * 2, pair * 2 + 1):
                            dc = half * 4 + j
                            dst = outs[:, j, :]
                            if j % 2 == 0:
                                nc.vector.tensor_scalar_add(
                                    dst, ps2[j][:], b2s[:, dc:dc + 1])
                            else:
                                nc.scalar.activation(
                                    dst, ps2[j][:], AFT.Identity,
                                    bias=b2s[:, dc:dc + 1])
                        d0 = half * 4 + pair * 2
                        nc.sync.dma_start(
                            yp_r[:, d0:d0 + 2, t0:t0 + TILE_N],
                            outs[:, pair * 2:pair * 2 + 2, :])

            # Warm-up: dummy matmuls on zeroed tiles fill the otherwise
            # idle startup-DMA window so the PE's activity monitor (HAM)
            # reaches full clock before real work arrives.
            warm_w = w1pool.tile([128, 128], f16, tag="warmw")
            warm_x = xxpool.tile([128, 512], f16, tag="warmx")
            nc.gpsimd.memset(warm_w[:], 0.0)
            nc.gpsimd.memset(warm_x[:], 0.0)
            warm_ps = pspool.tile([128, 512], f32, tag="ps", name="warm_ps")
            for _ in range(WARMUP_MMS):
                nc.tensor.matmul(warm_ps[:], lhsT=warm_w[:], rhs=warm_x[:],
                                 start=True, stop=True)

            # head: tile-0 x loads per-chunk so the first matmul can start
            # as soon as chunk 0 lands (one 3D DMA would gate fc=0 on the
            # whole 1MB tile); w1 strips 0-5 hoisted; then spill x
            # tile-0 x chunks ride the otherwise-idle scalar/gpsimd DGE
            # queues in parallel with the w1 strips on the sync queue, so
            # the first stage-1 iteration is fed ~4us sooner; bias
            # constants follow on the same side queues
            xs = xpool.tile([128, DC, TILE_N], f16, tag="xs", name="xs_0")
            pre = {0: load_w1s(0)}
            for c in range(DC):
                # gpsimd first: the scalar queue opens late (blocked by
                # the ~1.3us activation-table load in the preamble)
                eng = (nc.gpsimd, nc.sync, nc.scalar)[c % 3]
                eng.dma_start(xs[:, c, :], xp_r[:, c, 0:TILE_N])
            nc.scalar.dma_start(b1s[:], b1p)
            nc.gpsimd.dma_start(b2s[:], b2p)
            nc.scalar.dma_start(b1xs[:], b1xp)
            nc.gpsimd.dma_start(b2xs[:], b2xp)
            for fc in range(1, 6):
                pre[fc] = load_w1s(fc)
            xsx = xxpool.tile([128, DC, SPILL_W], f16, tag="xsx")
            nc.sync.dma_start(xsx[:, :, :],
                              xp_r[:, :, MAIN_CAP:MAIN_CAP + SPILL_W])

            # background queue: spill stage-1 units (w1x strips prefetched
            # 2 ahead), then spill stage-2 group units (strips 2 groups
            # ahead); emitted during stage-1 of tiles 1-3
            bg.add(lambda: dma_w1x(0))
            bg.add(lambda: dma_w1x(1))
            for fc in range(FC):
                bg.add(spill_s1_unit(fc, xsx))
            bg.add(lambda: dma_w2x_group(0))
            bg.add(lambda: dma_w2x_group(1))
            for g in range(8):
                bg.add(spill_s2_unit(g))

            nxt = {}

            def hook_prefetch(i):
                def fx():
                    nxt['xs'] = load_xs(i * TILE_N)
                def fw():
                    nxt['pre'] = {0: load_w1s(0), 1: load_w1s(1)}
                return {8: fx, 16: fw}

            for i in range(N_TILES):
                t0 = i * TILE_N
                h = hpool.tile([128, FC * TILE_N], f16, tag="h",
                               name=f"h_{i}")
                stage1(xs, h, t0, pre=pre, bg_rate=0.0 if i == 0 else 0.5)
                if i == N_TILES - 1:
                    bg.flush()
                    spill_drain()
                    stage2_halves(h, t0)
                else:
                    stage2_full(h, t0, hooks=hook_prefetch(i + 1))
                    xs = nxt.pop('xs')
                    pre = nxt.pop('pre')

    nc.compile()
    return nc


def _ensure_trace_hook():
    """bass_utils' axon trace path needs antenv.axon_hooks; inject it."""
    import sys
    import types
    try:
        import antenv
        if "antenv.axon_hooks" in sys.modules:
            return
        from trn_agent_boot.trn_boot import _ntff_profile_via_ctypes
        mod = types.ModuleType("antenv.axon_hooks")
        hook = [_ntff_profile_via_ctypes("/opt/axon/libaxon_pjrt.so")]
        mod.set_axon_ntff_profile_hook = lambda h: hook.__setitem__(0, h)
        mod.get_axon_ntff_profile_hook = lambda: hook[0]
        sys.modules["antenv.axon_hooks"] = mod
        antenv.axon_hooks = mod
    except Exception:
        pass


def _route(xf, router_w, router_b):
    """Top-2 routing, bit-identical to the reference (jax on CPU)."""
    try:
        import jax
        import jax.numpy as jnp

        cpu = jax.devices("cpu")[0]
        with jax.default_device(cpu):
            logits = (jnp.asarray(xf) @ jnp.asarray(router_w)
                      + jnp.asarray(router_b))
            top_vals, top_idx = jax.lax.top_k(logits, TOP_K)
            wts = jax.nn.softmax(top_vals, axis=-1)
        return np.asarray(top_idx), np.asarray(wts, np.float32)
    except Exception:
        # numpy fallback; ties resolve to the lower index like lax.top_k
        logits = xf @ router_w + router_b
        order = np.argsort(-logits, axis=1, kind="stable")[:, :TOP_K]
        vals = np.take_along_axis(logits, order, axis=1)
        ex = np.exp(vals - vals.max(axis=1, keepdims=True))
        wts = (ex / ex.sum(axis=1, keepdims=True)).astype(np.float32)
        return order, wts


def _w1_layout(w):
    """[D_MODEL, D_FF] -> [FC, 128, D_MODEL] strips for lhsT slicing."""
    return np.ascontiguousarray(
        w.reshape(DC, 128, FC, 128).transpose(2, 1, 0, 3)
        .reshape(FC, 128, D_MODEL).astype(np.float16))


def kernel(x, router_w, router_b, w1, b1, w2, b2):
    global LAST_EXEC_NS, LAST_TRACE_PATH
    from concourse import bass_utils

    x = np.asarray(x, np.float32)
    router_w = np.asarray(router_w, np.float32)
    router_b = np.asarray(router_b, np.float32)
    w1 = np.asarray(w1, np.float32)
    b1 = np.asarray(b1, np.float32)
    w2 = np.asarray(w2, np.float32)
    b2 = np.asarray(b2, np.float32)

    orig_shape = x.shape
    xf = x.reshape(-1, x.shape[-1])
    T = xf.shape[0]

    top_idx, wts = _route(xf, router_w, router_b)

    tok_ids = []
    gates = []
    for e in range(N_EXPERTS):
        mask = top_idx == e                      # [T, K]
        sel = mask.any(axis=1)
        ids = np.nonzero(sel)[0]
        # each token picks distinct experts, so at most one k matches
        gk = np.where(mask[ids, 0], wts[ids, 0], wts[ids, 1]).astype(np.float32)
        tok_ids.append(ids)
        gates.append(gk)

    counts = np.array([len(i) for i in tok_ids])

    # spill chunks: tokens beyond MAIN_CAP, split into <=SPILL_W chunks,
    # one chunk per core
    spill_chunks = []           # (expert, ids, gates)
    for e in range(N_EXPERTS):
        rem = counts[e] - MAIN_CAP
        off = MAIN_CAP
        while rem > 0:
            n = min(rem, SPILL_W)
            spill_chunks.append((e, tok_ids[e][off:off + n],
                                 gates[e][off:off + n]))
            off += n
            rem -= n
    assert len(spill_chunks) <= N_CORES, (
        f"spill overflow: {len(spill_chunks)} chunks > {N_CORES} cores")

    key = "v3"
    if key not in _nc_cache:
        _nc_cache[key] = _build_nc()
    nc = _nc_cache[key]

    xfT = xf.astype(np.float16)
    w2_f16 = [np.ascontiguousarray(w2[e].astype(np.float16))
              for e in range(N_EXPERTS)]
    w1_lay = [_w1_layout(w1[e]) for e in range(N_EXPERTS)]
    in_maps = []
    for e in range(N_EXPERTS):
        ce = min(counts[e], MAIN_CAP)
        xpad = np.zeros((D_MODEL, CT), np.float16)
        xpad[:, :ce] = xfT[tok_ids[e][:ce]].T
        if e < len(spill_chunks):
            se, sids = spill_chunks[e][0], spill_chunks[e][1]
        else:
            se, sids = e, np.empty(0, np.int64)
        if len(sids):
            xpad[:, MAIN_CAP:MAIN_CAP + len(sids)] = xfT[sids].T
        in_maps.append({
            "xp": xpad,
            "w1p": w1_lay[e],
            "w2p": w2_f16[e],
            "w1x": w1_lay[se],
            "w2x": w2_f16[se],
            "b1p": np.ascontiguousarray(b1[e].reshape(FC, 128).T),
            "b2p": np.ascontiguousarray(b2[e].reshape(DC, 128).T),
            "b1xp": np.ascontiguousarray(b1[se].reshape(FC, 128).T),
            "b2xp": np.ascontiguousarray(b2[se].reshape(DC, 128).T),
        })

    if TRACE:
        _ensure_trace_hook()
    res = bass_utils.run_bass_kernel_spmd(
        nc, in_maps, core_ids=list(range(N_CORES)), trace=TRACE)
    LAST_EXEC_NS = res.exec_time_ns
    LAST_TRACE_PATH = (res.instructions_and_trace[1]
                       if res.instructions_and_trace else None)

    out = np.zeros((T, D_MODEL), np.float32)
    for e in range(N_EXPERTS):
        ye = np.asarray(res.results[e]["yp"]).astype(np.float32)  # [D, CT]
        ce = min(counts[e], MAIN_CAP)
        out[tok_ids[e][:ce]] += gates[e][:ce, None] * ye.T[:ce]
        if e < len(spill_chunks):
            se, sids, sg = spill_chunks[e]
            if len(sids):
                out[sids] += sg[:, None] * ye.T[MAIN_CAP:MAIN_CAP + len(sids)]

    return out.reshape(orig_shape)
